# revision 40
# baseline (speedup 1.0000x reference)
"""MoE expert-parallel kernel for Trainium2 (8 NeuronCores).

Strategy (DeepseekV2 experts, T=2048 H=2048 I=1408 E=16 K=6):
  - Expert parallel, 3 slots per core: the 8 least-loaded experts sit whole
    in one slot; the 8 most-loaded are each SPLIT into two token-pieces
    spread over the remaining 16 cells (see _plan_cells). Slot caps
    (652, 352, 332) put 1336 padded matmul columns per core vs 1356 for
    the best unsplit 2-slot layout (ideal total/8 = 1314). Weights of
    split experts are loaded twice; DMA has the headroom.
  - Host (the shard/unshard step): compute combine weights, gather each
    cell's routed tokens into a zero-padded [cap, H] buffer, transpose to
    feature-major, cast to bf16.
  - Device (per core, per slot): GEMM1 (gate+up) -> silu*up -> GEMM2
    (down), all bf16 matmuls with fp32 PSUM accumulation, bf16 outputs.
    Weights stream HBM->SBUF in ~1MB slabs, tokens stay resident. The
    program front interleaves the first weight slab with the token tile in
    fine chunks (PE starts ~2us in) behind a short burst of dummy warm-up
    matmuls that walk the PE clock ramp during the DMA lead-in.
  - Host: transpose back, scale by combine weights, scatter-add into the
    full [T, H] output.

bf16 PE roofline: 528 col-cycles per routed token-column = 293.9us at
2.4GHz (TimelineSim books 305.7us with DMA lead-in/drain; earlier
octile-layout baseline booked 323.4us, 274.2us measured by the grading
profile). PE rate ~207ns per [128x128]x[128x512] bf16 MM, LDW hidden.
fp8 (DoubleRow) would double PE rate but e4m3's 3 mantissa bits put
~5% on the output -- over the 2e-2 gate.
"""

import os
import numpy as np
import ml_dtypes
from einops import rearrange

P = 128
N_CORES = 8

_BF16 = ml_dtypes.bfloat16

# Populated by kernel() when tracing is enabled (BASS_MOE_TRACE=1).
LAST_EXEC_TIME_NS = None
LAST_RESULTS = None

_PROGRAM_CACHE = {}


def _build_program(EPC, H, I, caps, reps=1, psum_cfg=(3, 3, 2), w1_bufs=4,
                   w2_bufs=3, subs_inner=False, out_bf16=True,
                   unify_psum=False, interleave_lead=True, w2_split=2,
                   warmup_mm=12, dual_queue=False, lead_xq=False,
                   tail_yq=False, hslab=512):
    """Build the per-core Bass/Tile program (same NEFF for all 8 cores).

    caps: per-slot token capacity (len EPC == slots per core). Each slot
    hosts one expert token-piece per _plan_cells; smaller slots do fewer
    matmul columns.

    Inputs (per core), all bf16:
      w1: [EPC, IT, P, KT, 2P]   w1[e,i,p,kt,j2] = gate_up.T tile; j2<128 gate, else up
      w2: [EPC, NS, P, IT, HSLAB] w2[e,s,p,it,hc] = down[h'=s*HSLAB+hc, i=it*128+p]
      xt{s}: [P, KT, caps[s]]     xt{s}[p,kt,t] = x_gathered[t, kt*128+p]
    Output:
      y{s}:  [HT, P, caps[s]] bf16 (f32 if not out_bf16)
             y{s}[ht,p,t] = out[t, h'=ht*128+p]
    """
    import concourse.bass as bass  # noqa: F401
    import concourse.tile as tile
    from concourse import bacc, mybir

    bf16 = mybir.dt.bfloat16
    f32 = mybir.dt.float32

    KT = H // P          # contraction tiles for GEMM1 (over H)
    IT = I // P          # contraction tiles for GEMM2 (over I)
    HT = H // P          # output tiles (over H)
    HSLAB = min(hslab, H)  # GEMM2 weight slab width (h' columns)
    NS = H // HSLAB      # number of GEMM2 slabs
    HQ = HSLAB // P      # 128-col groups per GEMM2 slab

    def make_subs(cap):
        subs = []
        off = 0
        while off < cap:
            w = min(512, cap - off)
            subs.append((off, w))
            off += w
        return subs

    slot_subs = [make_subs(c) for c in caps]
    cap_max = max(caps)

    nc = bacc.Bacc("TRN2", target_bir_lowering=False, debug=False,
                   num_devices=N_CORES)

    w1 = nc.dram_tensor("w1", [EPC, IT, P, KT, 2 * P], bf16, kind="ExternalInput")
    w2 = nc.dram_tensor("w2", [EPC, NS, P, IT, HSLAB], bf16, kind="ExternalInput")
    xt_d = [nc.dram_tensor(f"xt{s}", [P, KT, caps[s]], bf16,
                           kind="ExternalInput") for s in range(EPC)]
    y_d = [nc.dram_tensor(f"y{s}", [HT, P, caps[s]],
                          bf16 if out_bf16 else f32, kind="ExternalOutput")
           for s in range(EPC)]

    with tile.TileContext(nc) as tc:
        with (
            tc.tile_pool(name="xp", bufs=2) as xpool,
            tc.tile_pool(name="w1p", bufs=w1_bufs) as w1pool,
            tc.tile_pool(name="w2p", bufs=w2_bufs) as w2pool,
            tc.tile_pool(name="ap", bufs=2) as apool,
            tc.tile_pool(name="sp", bufs=3) as spool,
            tc.tile_pool(name="op", bufs=2) as opool,
            tc.tile_pool(name="ps", bufs=2, space="PSUM") as pspool,
        ):
            if warmup_mm:
                # dummy matmuls on a zeroed tile while the first weight/token
                # DMAs are in flight: walks the PE clock out of its cold-start
                # p-state ramp before the real work arrives. Sized to END
                # before the first real weights land (~3.3us) — overrunning
                # would push the real matmuls back.
                wz = spool.tile([P, 2 * P], bf16, name="warmz", tag="sig")
                nc.vector.memset(wz[:], 0.0)
                psw = pspool.tile([P, 2 * P], f32, name="pswarm", tag="psg",
                                  bufs=psum_cfg[0])
                for wi in range(warmup_mm):
                    lhs = wz[:, 0:P] if wi % 2 else wz[:, P:2 * P]
                    nc.tensor.matmul(psw[:], lhs, wz[:, 0:2 * P],
                                     start=True, stop=True)
            for rep in range(reps):
              for e0 in range(EPC):
                e = rep * EPC + e0  # unique tile names across reps
                cap = caps[e0]
                subs = slot_subs[e0]
                xts = xpool.tile([P, KT, cap_max], bf16, name=f"xts{e}",
                                 tag="xts")[:, :, :cap]
                lead = e == 0 and interleave_lead and KT % 4 == 0 and KT >= 8
                xq = nc.scalar if dual_queue or (lead and lead_xq) else nc.sync
                w1t0 = None
                if lead:
                    # Program start: the first GEMM1 matmul needs only
                    # w1[it=0, kt=0] and xt[kt=0]. Issue the it=0 weight slab
                    # and the token tile as fine chunks in lockstep so the PE
                    # starts ~2us in instead of waiting for whole tiles.
                    w1t0 = w1pool.tile([P, KT, 2 * P], bf16, name=f"w1t{e}_0",
                                       tag="w1t")
                    nc.sync.dma_start(w1t0[:, 0:2], w1[e0, 0, :, 0:2])
                    xq.dma_start(xts[:, 0:2, :], xt_d[e0][:, 0:2, :])
                    nc.sync.dma_start(w1t0[:, 2:4], w1[e0, 0, :, 2:4])
                    xq.dma_start(xts[:, 2:4, :], xt_d[e0][:, 2:4, :])
                    for c0 in range(4, KT, 4):
                        nc.sync.dma_start(w1t0[:, c0:c0 + 4],
                                          w1[e0, 0, :, c0:c0 + 4])
                        xq.dma_start(xts[:, c0:c0 + 4, :],
                                     xt_d[e0][:, c0:c0 + 4, :])
                else:
                    # split the token load so the first GEMM1 tiles don't
                    # wait for the whole transfer
                    KTC = 4 if KT % 4 == 0 else 1
                    step = KT // KTC
                    for ktc in range(0, KT, step):
                        xq.dma_start(xts[:, ktc:ktc + step, :],
                                     xt_d[e0][:, ktc:ktc + step, :])

                act = apool.tile([P, IT, cap_max], bf16, name=f"act{e}",
                                 tag="act")[:, :, :cap]

                # ---- GEMM1: out1 = x @ gate_up.T ; act = silu(gate) * up ----
                for it in range(IT):
                    if it == 0 and w1t0 is not None:
                        w1t = w1t0
                    else:
                        w1t = w1pool.tile([P, KT, 2 * P], bf16,
                                          name=f"w1t{e}_{it}", tag="w1t")
                        nc.sync.dma_start(w1t[:], w1[e0, it])
                    if subs_inner:
                        psgs = [pspool.tile([P, 512], f32,
                                            name=f"psg{e}_{it}_{off}", tag="psg",
                                            bufs=psum_cfg[0])
                                for off, wd in subs]
                        psus = [pspool.tile([P, 512], f32,
                                            name=f"psu{e}_{it}_{off}", tag="psu",
                                            bufs=psum_cfg[1])
                                for off, wd in subs]
                        for kt in range(KT):
                            st = kt == 0
                            sp = kt == KT - 1
                            for si, (off, wd) in enumerate(subs):
                                nc.tensor.matmul(psgs[si][:, :wd], w1t[:, kt, 0:P],
                                                 xts[:, kt, off:off + wd],
                                                 start=st, stop=sp)
                            for si, (off, wd) in enumerate(subs):
                                nc.tensor.matmul(psus[si][:, :wd],
                                                 w1t[:, kt, P:2 * P],
                                                 xts[:, kt, off:off + wd],
                                                 start=st, stop=sp)
                        sub_ps = list(zip(psgs, psus))
                    else:
                        sub_ps = None
                    for si, (off, wd) in enumerate(subs):
                        if sub_ps is not None:
                            psg, psu = sub_ps[si]
                        else:
                            psg = pspool.tile([P, 512], f32,
                                              name=f"psg{e}_{it}_{off}", tag="ps8" if unify_psum else "psg",
                                              bufs=8 if unify_psum else psum_cfg[0])
                            psu = pspool.tile([P, 512], f32,
                                              name=f"psu{e}_{it}_{off}", tag="ps8" if unify_psum else "psu",
                                              bufs=8 if unify_psum else psum_cfg[1])
                            for kt in range(KT):
                                st = kt == 0
                                sp = kt == KT - 1
                                nc.tensor.matmul(psg[:, :wd], w1t[:, kt, 0:P],
                                                 xts[:, kt, off:off + wd],
                                                 start=st, stop=sp)
                                nc.tensor.matmul(psu[:, :wd], w1t[:, kt, P:2 * P],
                                                 xts[:, kt, off:off + wd],
                                                 start=st, stop=sp)
                        sig = spool.tile([P, 512], f32, name=f"sig{e}_{it}_{off}",
                                         tag="sig")
                        nc.scalar.activation(sig[:, :wd], psg[:, :wd],
                                             mybir.ActivationFunctionType.Sigmoid)
                        sil = spool.tile([P, 512], f32, name=f"sil{e}_{it}_{off}",
                                         tag="sil")
                        nc.vector.tensor_mul(out=sil[:, :wd], in0=sig[:, :wd],
                                             in1=psg[:, :wd])
                        nc.vector.tensor_mul(out=act[:, it, off:off + wd],
                                             in0=sil[:, :wd], in1=psu[:, :wd])

                # ---- GEMM2: y = act @ down.T ----
                for s in range(NS):
                    w2t = w2pool.tile([P, IT, HSLAB], bf16, name=f"w2t{e}_{s}",
                                      tag="w2t")
                    if w2_split > 1 and IT >= w2_split:
                        # chunked so the slab's first ldweights doesn't wait
                        # for the full 1.4MB transfer
                        itc = IT // w2_split
                        for c0 in range(0, IT, itc):
                            c1 = min(c0 + itc, IT)
                            nc.sync.dma_start(w2t[:, c0:c1], w2[e0, s, :, c0:c1])
                    else:
                        nc.sync.dma_start(w2t[:], w2[e0, s])
                    # last slab of the last expert: per-hq output DMAs so the
                    # final copy->DMA chain pipelines (shorter exposed tail)
                    fine_out = (rep == reps - 1 and e0 == EPC - 1
                                and s == NS - 1)
                    for off, wd in subs:
                        ot = opool.tile([P, HQ, 512],
                                        bf16 if out_bf16 else f32,
                                        name=f"ot{e}_{s}_{off}", tag="ot")
                        for hq in range(HQ):
                            psy = pspool.tile([P, 512], f32,
                                              name=f"psy{e}_{s}_{hq}_{off}",
                                              tag="ps8" if unify_psum else "psy",
                                              bufs=8 if unify_psum else psum_cfg[2])
                            for it in range(IT):
                                nc.tensor.matmul(psy[:, :wd],
                                                 w2t[:, it, hq * P:(hq + 1) * P],
                                                 act[:, it, off:off + wd],
                                                 start=(it == 0), stop=(it == IT - 1))
                            nc.vector.tensor_copy(out=ot[:, hq, :wd],
                                                  in_=psy[:, :wd])
                            if fine_out:
                                # the kernel tail is SP-dispatch serialized;
                                # ACT's queue is idle during GEMM2
                                yq = nc.scalar if dual_queue or tail_yq \
                                    else nc.sync
                                yq.dma_start(
                                    y_d[e0][s * HQ + hq, :, off:off + wd],
                                    ot[:, hq, :wd])
                        if not fine_out:
                            yq = nc.scalar if dual_queue else nc.sync
                            yq.dma_start(
                                y_d[e0][s * HQ:(s + 1) * HQ, :, off:off + wd]
                                .rearrange("ht p t -> p ht t"),
                                ot[:, :, :wd])

    nc.compile()
    return nc


def _plan_cells(counts, tok_idx, n_cores, force_2slot=False):
    """Assign (expert, token-range) cells to n_cores x S slots.

    Returns (caps, cells): caps[s] = slot-s capacity, cells[c][s] =
    (expert, lo, hi) token-range of tok_idx[expert] (expert == -1: empty).

    3-slot plan: the n_cores smallest experts sit whole in slot 0 (cap =
    their max); the n_cores biggest are each split into two pieces over the
    slot-1/slot-2 cells, with (c1, c2) minimized by brute force. Splitting
    experts lets every core carry ~total/n_cores tokens, beating the 2-slot
    octile scheme's max+9th-max capacity; weights of split experts are
    loaded twice (DMA has slack). Falls back to the 2-slot octile scheme.
    """
    E = len(counts)
    order = np.argsort(-counts, kind="stable")

    def rnd(v):
        return max(P, int(-(-int(v) // 4)) * 4)

    if E == 2 * n_cores and not force_2slot:
        big = order[:n_cores]
        small = order[n_cores:]
        c0 = rnd(counts[small].max())
        bc = sorted([(int(counts[g]), int(g)) for g in big])[::-1]  # desc
        best = None
        hi = int(counts[big].max())
        for c1 in range(rnd((hi + 1) // 2), c0 + 1, 4):
            for c2 in range(P, c1 + 1, 4):
                if best is not None and c1 + c2 >= best[0]:
                    continue
                # greedy feasibility: pieces over 8 c1-cells + 8 c2-cells
                n1, n2 = n_cores, n_cores
                use = []
                ok = True
                for n, g in bc:
                    if n <= c2 and n2 > 0:
                        n2 -= 1; use.append((g, ("c2",)))
                    elif n <= c1 and n1 > 0:
                        n1 -= 1; use.append((g, ("c1",)))
                    elif n <= c1 + c2 and n1 > 0 and n2 > 0:
                        n1 -= 1; n2 -= 1; use.append((g, ("c1", "c2")))
                    elif n <= 2 * c2 and n2 >= 2:
                        n2 -= 2; use.append((g, ("c2", "c2")))
                    elif n <= 2 * c1 and n1 >= 2:
                        n1 -= 2; use.append((g, ("c1", "c1")))
                    else:
                        ok = False
                        break
                if ok:
                    best = (c1 + c2, c1, c2, use)
        if best is not None and best[1] + best[2] + c0 < \
                rnd(counts[order[0]]) + rnd(counts[order[n_cores]]):
            _, c1, c2, use = best
            caps = (c0, c1, c2)
            cells = [[None] * 3 for _ in range(n_cores)]
            for i, g in enumerate(small):
                cells[i][0] = (int(g), 0, int(counts[g]))
            free = {1: list(range(n_cores)), 2: list(range(n_cores))}
            cap_of = {1: c1, 2: c2}
            for g, kinds in use:
                lo = 0
                for kind in kinds:
                    s = 1 if kind == "c1" else 2
                    take = min(cap_of[s], int(counts[g]) - lo)
                    cells[free[s].pop()][s] = (g, lo, lo + take)
                    lo += take
            for c in range(n_cores):
                for s in range(3):
                    if cells[c][s] is None:
                        cells[c][s] = (-1, 0, 0)
            return caps, cells

    # fallback: 2-slot octile scheme (any E divisible by n_cores)
    EPC = E // n_cores
    assign = order.reshape(EPC, n_cores).T
    caps = tuple(rnd(counts[assign[:, s]].max()) for s in range(EPC))
    cells = [[(int(assign[c, s]), 0, int(counts[assign[c, s]]))
              for s in range(EPC)] for c in range(n_cores)]
    return caps, cells


def _prep_host(hidden_states, top_k_index, top_k_weights, gate_up_proj,
               down_proj, slot_order_mode="desc", force_2slot=False,
               hslab=512):
    """Routing + per-core input construction (the shard step)."""
    hs = np.asarray(hidden_states, dtype=np.float32)
    idx = np.asarray(top_k_index)
    tkw = np.asarray(top_k_weights, dtype=np.float32)
    gup = np.asarray(gate_up_proj, dtype=np.float32)
    dwn = np.asarray(down_proj, dtype=np.float32)

    T, H = hs.shape
    E, I2, _ = gup.shape
    I = I2 // 2
    assert H % P == 0 and I % P == 0 and E % N_CORES == 0

    # combine[t, e] = sum of top_k_weights over slots routed to e
    combine = np.zeros((T, E), np.float32)
    np.add.at(combine, (np.arange(T)[:, None], idx), tkw)
    mask = np.zeros((T, E), bool)
    mask[np.arange(T)[:, None], idx] = True

    tok_idx = [np.nonzero(mask[:, e])[0] for e in range(E)]
    counts = np.array([len(t) for t in tok_idx])

    caps, cells = _plan_cells(counts, tok_idx, N_CORES, force_2slot=force_2slot)
    # slot processing order trades slot0's it=0 DMA exposure against w1
    # prefetch slack for the later slots
    slot_order = np.argsort([-c for c in caps], kind="stable")
    if slot_order_mode == "mid" and len(caps) == 3:
        # middle-sized slot first, biggest second: the big slot's tokens
        # stream during the first slot's compute
        slot_order = np.array([int(np.argsort(caps)[1]),
                               int(np.argmax(caps)), int(np.argmin(caps))])
    elif slot_order_mode == "asc":
        slot_order = np.argsort(caps, kind="stable")
    caps = tuple(caps[s] for s in slot_order)
    cells = [[cells[c][s] for s in slot_order] for c in range(N_CORES)]
    EPC = len(caps)

    hs_b = hs.astype(_BF16)
    HSLAB = min(hslab, H)

    w1z = None
    w2z = None
    in_maps = []
    for c in range(N_CORES):
        w1c = np.empty((EPC, I // P, P, H // P, 2 * P), _BF16)
        w2c = np.empty((EPC, H // HSLAB, P, I // P, HSLAB), _BF16)
        m = {"w1": w1c, "w2": w2c}
        for s in range(EPC):
            g, lo, hi = cells[c][s]
            if g < 0:
                if w1z is None:
                    w1z = np.zeros(w1c.shape[1:], _BF16)
                    w2z = np.zeros(w2c.shape[1:], _BF16)
                w1c[s] = w1z
                w2c[s] = w2z
                m[f"xt{s}"] = np.zeros((P, H // P, caps[s]), _BF16)
                continue
            gate = gup[g, :I]      # [I, H]
            up = gup[g, I:]        # [I, H]
            w1c[s, ..., :P] = rearrange(
                gate.astype(_BF16), "(i jg) (kt p) -> i p kt jg", jg=P, p=P)
            w1c[s, ..., P:] = rearrange(
                up.astype(_BF16), "(i jg) (kt p) -> i p kt jg", jg=P, p=P)
            w2c[s] = rearrange(
                dwn[g].astype(_BF16), "(ns hc) (it p) -> ns p it hc",
                hc=HSLAB, p=P)
            xtc = np.zeros((P, H // P, caps[s]), _BF16)
            tk = tok_idx[g][lo:hi]
            if len(tk):
                xtc[:, :, :len(tk)] = rearrange(
                    hs_b[tk], "t (kt p) -> p kt t", p=P)
            m[f"xt{s}"] = xtc
        in_maps.append(m)

    meta = dict(T=T, H=H, I=I, E=E, EPC=EPC, caps=caps, cells=cells,
                tok_idx=tok_idx, combine=combine)
    return in_maps, meta


def kernel(hidden_states, top_k_index, top_k_weights, gate_up_proj, down_proj):
    global LAST_EXEC_TIME_NS, LAST_RESULTS
    from concourse.bass_utils import run_bass_kernel_spmd

    in_maps, meta = _prep_host(hidden_states, top_k_index, top_k_weights,
                               gate_up_proj, down_proj)
    EPC, H, I, caps = meta["EPC"], meta["H"], meta["I"], meta["caps"]

    key = (EPC, H, I, caps)
    if key not in _PROGRAM_CACHE:
        _PROGRAM_CACHE[key] = _build_program(EPC, H, I, caps)
    nc = _PROGRAM_CACHE[key]

    trace = bool(int(os.environ.get("BASS_MOE_TRACE", "0")))
    res = None
    for attempt in range(3):
        try:
            res = run_bass_kernel_spmd(nc, in_maps, list(range(N_CORES)),
                                       trace=trace and attempt == 0)
            break
        except ModuleNotFoundError:
            # axon client without the NTFF profile hook package — run untraced
            trace = False
        except Exception as exc:  # transient NRT_EXEC_UNIT_UNRECOVERABLE flakes
            if attempt == 2 or "UNRECOVERABLE" not in str(exc).upper() \
                    and "UNAVAILABLE" not in str(exc).upper():
                raise
            import time
            import jax
            jax.clear_caches()
            time.sleep(2.0)
    LAST_EXEC_TIME_NS = res.exec_time_ns
    LAST_RESULTS = res

    T, E = meta["T"], meta["E"]
    combine, tok_idx, cells = meta["combine"], meta["tok_idx"], meta["cells"]
    out = np.zeros((T, H), np.float32)
    for c in range(N_CORES):
        for s in range(EPC):
            g, lo, hi = cells[c][s]
            if g < 0 or hi <= lo:
                continue
            tk = tok_idx[g][lo:hi]
            yc = res.results[c][f"y{s}"]  # [HT, P, caps[s]] bf16
            yt = rearrange(yc[:, :, :len(tk)].astype(np.float32),
                           "ht p t -> t (ht p)")
            out[tk] += yt * combine[tk, g][:, None]
    return out.astype(np.asarray(hidden_states).dtype)

